# revision 40
# baseline (speedup 1.0000x reference)
"""PointPillarScatter on 8 TRN2 cores via PE one-hot matmul.

Scatter -> dense-matmul transform: host packs pillars (sorted by output
column) into 32-slot windows per 128-column tile.  On device, one-hot
matrices P[slot, col] = (colof[slot] == col) are built 24 tiles at a
time with a single Vector tensor_tensor is_equal over a [96, 8*128]
tile (iota constant vs per-slot column offset broadcast; empty slots
get -1 so their row is all-zero), then PSUM[col, feat] = P^T @ feats
gives every output element exactly once.  P and feats are bf16 (the
one-hot matmul picks single bf16 values, no accumulation error; only
feats quantization ~2^-8 relative).

Layout: tile t uses slot partitions [32*(t%3), 32*(t%3)+32) and feats/
colof column t//3 -- matmul operands may sit at partition bases 0/32/64
(96-partition operand space, verified on HW; base 96 is rejected).
Each group of 4 tiles fills the 4 banks of one half of a persistent
8-bank PSUM tile at free offsets {0,512,1024,1536} (one accumulation
group per bank, out at bank offset 0), and a single strided-AP copy
[128, 4, 64] drains the group to a bf16 stage (values are already
bf16-quantized, so the narrowing is lossless; the host upcasts).  The
drain runs on Vector for g%4==2 and Scalar otherwise, so consecutive
same-PSUM-half drains alternate engines; each P-build tensor_tensor is
issued at a g%16==4 slot so it executes in Vector's copy gaps instead
of delaying them.

Sharding: core k owns flat output columns [k*88000, (k+1)*88000) of the
5*140800 (cav, y, x) space; 688 tiles of 128 cols per core.  Stage
tiles [128, 4096] collect 16 groups and drain to HBM in half-stage
DMAs, with the last 8 groups trickled out singly to shorten the tail.
Host re-assembles [5, 64, 200, 704].  Measured ~96.6us on HW (baseline
one-bank-per-matmul fp32 version: 192us).
"""

import numpy as np
import ml_dtypes

import concourse.bass as bass
import concourse.tile as tile
from concourse import mybir
from concourse.bass_utils import run_bass_kernel_spmd

NUM_FEATURES = 64
MAX_CAV = 5
NX, NY = 704, 200
NUM_PIXELS = NY * NX            # 140800
TOTAL = MAX_CAV * NUM_PIXELS    # 704000
N_CORES = 8
CORE_COLS = TOTAL // N_CORES    # 88000 flat columns per core
TILE_COLS = 128
N_TILES = 688                   # 688*128 = 88064 >= 88000
SLOTS = 32                      # max pillars per tile (seed-0 max is 23)
TRIB = 232                      # feats/colof columns (= ceil(688/3) padded to 8)
GROUPS = N_TILES // 4           # 172: 4 tiles -> 4 PSUM banks per group
GP_DMA = 16                     # groups per stage tile (16*256 = 4096 cols)
PBLK = 8                        # colof columns per P-build tensor_tensor
TAIL = 4                        # trickle out the last TAIL groups singly
OUT_W = N_TILES * NUM_FEATURES  # 44032

_PROG = None


def _split_excess_waits(nc, max_waits=1, mm_waits=1):
    """Walrus enforces tight per-instruction sync-wait encoding limits. Spill
    surplus waits onto single-wait EventSemaphore nops inserted just before
    the offending instruction on the same engine queue (same semantics:
    engine blocks at the nop, then proceeds).  Matmuls encode up to
    mm_waits waits; everything else gets max_waits."""
    for blk in nc.main_func.blocks:
        i = 0
        while i < len(blk.instructions):
            inst = blk.instructions[i]
            si = inst.sync_info
            lim = mm_waits if isinstance(inst, mybir.InstMatmult) else max_waits
            if si is None or len(si.on_wait) <= lim:
                i += 1
                continue
            waits = list(si.on_wait)
            keep, spill = waits[-lim:], waits[:-lim]
            for w in spill:
                nop = mybir.InstEventSemaphore(
                    name=f"I-{nc.next_id()}", ins=[], outs=[]
                )
                nop.engine = inst.engine
                nop.sync_info = mybir.SyncInfo(on_wait=[w], on_update=[])
                nc.register_instruction(nop)
                blk.instructions.insert(i, nop)
                i += 1
            si.on_wait = keep
            inst.sync_info = si
            i += 1


def _build_prog():
    f32 = mybir.dt.float32
    bf16 = mybir.dt.bfloat16
    nc = bass.Bass()
    # feats: tile t lives at partitions [32*(t%3), +32), free [64*(t//3), +64)
    feats = nc.dram_tensor("feats", [96, TRIB * 64], bf16, kind="ExternalInput")
    # colof and iota packed into one tensor: one descriptor-gen on the
    # critical path to the first P-build instead of two
    meta = nc.dram_tensor("meta", [96, TRIB + PBLK * 128], bf16,
                          kind="ExternalInput")
    # out[p, t*64+f] = feature f of tile t's column p
    out = nc.dram_tensor("out", [128, OUT_W], bf16, kind="ExternalOutput")

    with tile.TileContext(nc) as tc:
        with (
            tc.tile_pool(name="const", bufs=1) as constp,
            tc.tile_pool(name="pmat", bufs=6) as pmatp,
            tc.tile_pool(name="psump", bufs=1, space="PSUM") as psump,
            tc.tile_pool(name="stage", bufs=5) as stagep,
        ):
            meta_sb = constp.tile([96, TRIB + PBLK * 128], bf16)
            feats_sb = constp.tile([96, TRIB * 64], bf16)
            # descgen order matters (serial on the Sync sequencer): meta
            # prefix (colof + iota block 0) first, then feats chunk 0 --
            # together they unblock the first P-build AND the first matmuls
            mcut = TRIB + 256
            nc.sync.dma_start(meta_sb[:, 0:mcut], meta[:, 0:mcut])
            nc.sync.dma_start(feats_sb[:, 0:512], feats[:, 0:512])
            nc.sync.dma_start(meta_sb[:, mcut:], meta[:, mcut:])
            colof_sb = meta_sb[:, 0:TRIB]
            iota_sb = meta_sb[:, TRIB:TRIB + PBLK * 128]
            # geometric chunks: tiny first loads unblock the first matmuls fast
            edges_c3 = [8, 16, 32, 64, 120, 176, 232]
            for a, b in zip(edges_c3[:-1], edges_c3[1:]):
                nc.sync.dma_start(
                    feats_sb[:, a * 64:b * 64], feats[:, a * 64:b * 64])
            # persistent 8-bank PSUM tile; groups rotate over its two halves
            psum = psump.tile([128, 4096], f32, space="PSUM")

            # schedule each P-build's issue group inside an ACT-copy stretch
            # (g % 16 == 2) so the tensor_tensor never sits between two DVE
            # group-copies in the Vector FIFO and delays them
            n_pb = (TRIB + PBLK - 1) // PBLK
            tt_issue = {}
            for pb in range(n_pb):
                fg = min(6 * pb, GROUPS - 1)
                base = (fg // 16) * 16 + 7
                if base > fg:
                    base -= 16
                tt_issue.setdefault(max(base, 0), []).append(pb)

            P8s = {}
            st = None
            st_col0 = 0
            dma_done = 0
            for g in range(GROUPS):
                for pb in tt_issue.get(g, ()):
                    P8s[pb] = pmatp.tile([96, PBLK * 128], bf16, name="P8")
                    c0 = pb * PBLK
                    if pb == 0:
                        # split into 2-col pieces: each ramp group waits only
                        # on the piece covering its own tiles
                        for lo, hi in ((0, 2), (2, 4), (4, 6), (6, PBLK)):
                            nc.vector.tensor_tensor(
                                out=P8s[pb][:, lo * 128:hi * 128].rearrange(
                                    "p (a b) -> p a b", a=hi - lo),
                                in0=iota_sb[:, 0:(hi - lo) * 128].rearrange(
                                    "p (a b) -> p a b", a=hi - lo),
                                in1=colof_sb[:, lo:hi].unsqueeze(2)
                                    .to_broadcast([96, hi - lo, 128]),
                                op=mybir.AluOpType.is_equal,
                            )
                    else:
                        nc.vector.tensor_tensor(
                            out=P8s[pb][:].rearrange("p (a b) -> p a b", a=PBLK),
                            in0=iota_sb.rearrange("p (a b) -> p a b", a=PBLK),
                            in1=colof_sb[:, c0:c0 + PBLK].unsqueeze(2)
                                .to_broadcast([96, PBLK, 128]),
                            op=mybir.AluOpType.is_equal,
                        )
                if g % GP_DMA == 0:
                    st = stagep.tile([128, GP_DMA * 256], bf16)
                    st_col0 = g * 256
                ps0 = (g % 2) * 2048
                for j in range(4):
                    t = 4 * g + j
                    b3, c3 = t % 3, t // 3
                    bb = c3 % PBLK
                    P8 = P8s[c3 // PBLK]
                    nc.tensor.matmul(
                        out=psum[:, ps0 + j * 512:ps0 + j * 512 + 64],
                        lhsT=P8[32 * b3:32 * b3 + 32, bb * 128:(bb + 1) * 128],
                        rhs=feats_sb[32 * b3:32 * b3 + 32, c3 * 64:(c3 + 1) * 64],
                        start=True,
                        stop=True,
                        skip_group_check=True,
                    )
                src = psum[:, ps0:ps0 + 2048].rearrange(
                    "p (a b) -> p a b", a=4)[:, :, 0:64]
                col0 = g * 256 - st_col0
                dst = st[:, col0:col0 + 256].rearrange("p (a b) -> p a b", a=4)
                if g % 4 == 2 and g >= 8:
                    nc.vector.tensor_copy(dst, src)
                else:
                    nc.scalar.activation(
                        dst, src, mybir.ActivationFunctionType.Copy)
                # trickle out the tail so the last DMA is small
                end_col = (g + 1) * 256
                if (g % 8 == 7 or g == GROUPS - 1
                        or (g >= GROUPS - TAIL and end_col > dma_done)):
                    nc.sync.dma_start(
                        out[:, dma_done:end_col],
                        st[:, dma_done - st_col0:end_col - st_col0])
                    dma_done = end_col
    _split_excess_waits(nc)
    return nc


def _host_prep(voxel_coords, pillar_features):
    vc = voxel_coords.astype(np.int64)
    flat = vc[:, 0] * NUM_PIXELS + vc[:, 2] * NX + vc[:, 3]
    feats = np.ascontiguousarray(pillar_features, dtype=np.float32)
    core = flat // CORE_COLS
    rem = flat - core * CORE_COLS
    t = rem // TILE_COLS
    cof = rem - t * TILE_COLS
    b3 = t % 3
    c3 = t // 3
    # slot = rank of pillar within its (core, tile) group
    order = np.argsort(flat, kind="stable")
    gid_sorted = (core * N_TILES + t)[order]
    rank_sorted = np.arange(len(flat)) - np.searchsorted(
        gid_sorted, gid_sorted, side="left"
    )
    slot = np.empty(len(flat), np.int64)
    slot[order] = rank_sorted
    assert slot.max() < SLOTS, f"tile overflow: {slot.max() + 1} slots"
    row = 32 * b3 + slot

    iota_arr = np.broadcast_to(
        np.tile(np.arange(128, dtype=np.float32), PBLK), (96, PBLK * 128)
    ).astype(np.float32)
    in_maps = []
    for cidx in range(N_CORES):
        m = core == cidx
        fa = np.zeros((96, TRIB, 64), np.float32)
        ca = np.full((96, TRIB), -1.0, np.float32)
        ca[row[m], c3[m]] = cof[m]
        fa[row[m], c3[m], :] = feats[m]
        meta = np.concatenate([ca, iota_arr], axis=1)
        in_maps.append({
            "feats": fa.reshape(96, TRIB * 64).astype(ml_dtypes.bfloat16),
            "meta": meta.astype(ml_dtypes.bfloat16),
        })
    return in_maps


def _unshard(core_outs):
    full = np.empty((TOTAL, NUM_FEATURES), np.float32)
    for cidx, o in enumerate(core_outs):       # o: [128, OUT_W] bf16
        o = np.asarray(o).astype(np.float32)
        r = o.reshape(128, N_TILES, 64).transpose(1, 0, 2)
        r = r.reshape(N_TILES * 128, 64)
        full[cidx * CORE_COLS:(cidx + 1) * CORE_COLS] = r[:CORE_COLS]
    return np.ascontiguousarray(
        full.reshape(MAX_CAV, NUM_PIXELS, NUM_FEATURES)
        .transpose(0, 2, 1)
        .reshape(MAX_CAV, NUM_FEATURES, NY, NX)
    )


def kernel(voxel_coords, pillar_features):
    global _PROG
    if _PROG is None:
        _PROG = _build_prog()
    in_maps = _host_prep(voxel_coords, pillar_features)
    res = run_bass_kernel_spmd(_PROG, in_maps, list(range(N_CORES)))
    return _unshard([r["out"] for r in res.results])
